# revision 15
# baseline (speedup 1.0000x reference)
"""Block-circulant linear (MINI_BLOCK=4) via length-4 rFFT factorization on 8 trn2 cores.

Math: out = x @ W^T where W[4y+n, 4x+j] = eigens[y, x, (n-j) mod 4].
In the length-4 DFT domain the circulant contraction factors into 5 real
matmul chains over the block-index axis gx=1024 (Gauss 3-mult for the complex
bin; ~3.2x fewer FLOPs than the dense 4096^3 matmul):
  X0 = x0+x1+x2+x3, X1 = (x0-x2) + i(x3-x1), X2 = x0-x1+x2-x3  (per block of 4)
  Y0 = X0 E0, Y2 = X2 E2
  Gauss (combos on the X side so only ONE derived E matrix is needed):
    g1 = X1r(E1r+E1i), g2 = (X1i-X1r)E1r, g3 = (X1r+X1i)E1i
    Y1r = g1-g3, Y1i = g1+g2
  o0 = Y0+Y1r+Y2, o1 = Y0-Y1i-Y2, o2 = Y0-Y1r+Y2, o3 = Y0+Y1i-Y2  (scales in E)

Device does ONLY the five matmul chains plus cheap DVE adds; both DFT
butterflies are data-independent linear prep on the host. Transport is
minimized (4 x-streams s02/s13/X1r/X1i, 4 E matrices, Es derived on-device;
8 MB yc0-critical, 12 MB in / 4 MB out total) and, because every engine
queue executes strictly in order, scheduled so consumption matches arrival:
 - x is shipped packed per b-tile ([s, bt, p, xc, b], one contiguous 256 KB
   DMA each) so tile k of yc0 is runnable after ~(5+k) MB instead of 8 MB;
 - the scalar engine issues NO input DMAs (its PSUM-drain copies would queue
   behind them); inputs ride sync+gpsimd, outputs ride scalar;
 - E-yc1 is issued behind the yc0 bytes on the same queues (prefetch into
   double-buffered tiles) so yc1 never waits.
Tensor engine: 320x 128x128x512 bf16 matmuls at 216 ns sustained = 69 us.

Sharding: data-parallel over batch, 512 rows per core; E replicated.
bf16 operands, fp32 PSUM; Y-streams returned bf16 (adds ~5e-4 rel err).
"""
import numpy as np

B, IN, OUT, BLK = 4096, 4096, 4096, 4
GX, GY = IN // BLK, OUT // BLK        # 1024, 1024
NCORES = 8
BS = B // NCORES                      # 512 batch rows per core
BT = BS // 128                        # 4 b-tiles
XC = GX // 128                        # 8 x-chunks (contraction)
YCS = 512                             # y-chunk size (matmul N)
YCN = GY // YCS                       # 2 y-chunks

_cache = {}


def _build_nc():
    from concourse import bacc
    import concourse.mybir as mybir
    from concourse.tile import TileContext

    f32 = mybir.dt.float32
    bf16 = mybir.dt.bfloat16

    nc = bacc.Bacc("TRN2", target_bir_lowering=False, debug=False,
                   enable_asserts=False, num_devices=NCORES)
    # 4 host-prepped x streams (s02, s13, X1r, X1i), packed per b-tile
    xs_d = nc.dram_tensor("xs", [4, BT, 128, XC, 128], bf16, kind="ExternalInput")
    # 4 E matrices; Es = E1r+E1i is derived on-device
    e_d = [nc.dram_tensor(nm, [YCN, XC, 128, YCS], bf16, kind="ExternalInput")
           for nm in ("e0", "e1r", "e1i", "e2")]
    # 4 Y streams out: Y0, Y2, Y1r, Y1i (host applies the inverse butterfly)
    ys_d = nc.dram_tensor("ys", [4, BS, GY], bf16, kind="ExternalOutput")

    with TileContext(nc) as tc:
        with (
            tc.tile_pool(name="xt", bufs=1) as xtp,
            tc.tile_pool(name="epool", bufs=2) as ep,
            tc.tile_pool(name="vpool", bufs=2) as vp,
            tc.tile_pool(name="outp", bufs=3) as op_,
            tc.tile_pool(name="mpsum", bufs=1, space="PSUM") as mps,
        ):
            # per-(stream, bt) tiles; free layout [xc, b] is contiguous 2 KB
            # per partition so each load is one big-segment DMA
            xin = [[xtp.tile([128, XC, 128], bf16, tag=f"xi{s}b{bt}",
                             name=f"xi{s}b{bt}") for bt in range(BT)]
                   for s in range(4)]   # s02, s13, X1r, X1i
            xdrv = [[xtp.tile([128, XC, 128], bf16, tag=f"xd{s}b{bt}",
                              name=f"xd{s}b{bt}") for bt in range(BT)]
                    for s in range(4)]  # X0, X2, Xd=X1i-X1r, X1s=X1r+X1i

            def load_x(bt, halves=False):
                # halves=True splits each stream load at xc=4 so tile bt's
                # first xc-groups unblock after half the bytes (ramp only)
                cuts = ((0, 4), (4, 8)) if halves else ((0, 8),)
                for lo, hi in cuts:
                    xsl = slice(lo, hi)
                    nc.sync.dma_start(out=xin[0][bt][:, xsl], in_=xs_d[0, bt][:, xsl])
                    nc.gpsimd.dma_start(out=xin[1][bt][:, xsl], in_=xs_d[1, bt][:, xsl])
                    nc.gpsimd.dma_start(out=xin[2][bt][:, xsl], in_=xs_d[2, bt][:, xsl])
                    nc.sync.dma_start(out=xin[3][bt][:, xsl], in_=xs_d[3, bt][:, xsl])
                    # forward butterfly (bf16, on the otherwise-idle DVE)
                    nc.vector.tensor_add(out=xdrv[0][bt][:, xsl], in0=xin[0][bt][:, xsl], in1=xin[1][bt][:, xsl])
                    nc.vector.tensor_sub(out=xdrv[1][bt][:, xsl], in0=xin[0][bt][:, xsl], in1=xin[1][bt][:, xsl])
                    nc.vector.tensor_sub(out=xdrv[2][bt][:, xsl], in0=xin[3][bt][:, xsl], in1=xin[2][bt][:, xsl])
                    nc.vector.tensor_add(out=xdrv[3][bt][:, xsl], in0=xin[2][bt][:, xsl], in1=xin[3][bt][:, xsl])

            def e_tiles():
                return [ep.tile([128, XC, YCS], bf16, tag=f"e{k}", name=f"et{k}")
                        for k in range(5)]  # E0, E1r, E1i, E2, Es(derived)

            def e_load(et, yc, h):
                hs = slice(4 * h, 4 * h + 4)
                srcs = [e_d[k][yc].rearrange("c p y -> p c y")[:, hs]
                        for k in range(4)]
                nc.gpsimd.dma_start(out=et[0][:, hs], in_=srcs[0])
                nc.sync.dma_start(out=et[3][:, hs], in_=srcs[3])
                nc.gpsimd.dma_start(out=et[1][:, hs], in_=srcs[1])
                nc.sync.dma_start(out=et[2][:, hs], in_=srcs[2])
                # Es = E1r + E1i
                nc.vector.tensor_add(out=et[4][:, hs], in0=et[1][:, hs],
                                     in1=et[2][:, hs])

            # Queue programs (in-order per engine): the first xc-group of
            # tile0 needs ALL five E-h0 halves + x-b0, so E-h0s go absolutely
            # first; then x-b0 (halved), E-h1s, x-b1..3, E-yc1 prefetch.
            et0 = e_tiles()
            etn = e_tiles()
            e_load(et0, 0, 0)
            load_x(0, halves=True)
            e_load(et0, 0, 1)
            load_x(1)
            load_x(2)
            load_x(3)
            e_load(etn, 1, 0)
            e_load(etn, 1, 1)

            for yc in range(YCN):
                et = et0 if yc == 0 else etn
                for bt in range(BT):
                    # chains: y0=X0*E0, y2=X2*E2, g2=Xd*E1r, g3=X1s*E1i,
                    # g1=X1r*Es (Es derived on-device, so g1 runs last).
                    # bufs=2 on the first three chains hides drain latency;
                    # g3/g1 (bufs=1) are freed within ~0.7us of tile end by
                    # the scalar g3-copy / DVE g1-copy.
                    y0 = mps.tile([128, YCS], f32, tag="y0", bufs=2)
                    y2 = mps.tile([128, YCS], f32, tag="y2", bufs=2)
                    g2 = mps.tile([128, YCS], f32, tag="g2", bufs=2)
                    g3 = mps.tile([128, YCS], f32, tag="g3")
                    g1 = mps.tile([128, YCS], f32, tag="g1")
                    for xc in range(XC):
                        st, sp = xc == 0, xc == XC - 1
                        nc.tensor.matmul(y0, xdrv[0][bt][:, xc], et[0][:, xc], start=st, stop=sp)
                        nc.tensor.matmul(y2, xdrv[1][bt][:, xc], et[3][:, xc], start=st, stop=sp)
                        nc.tensor.matmul(g2, xdrv[2][bt][:, xc], et[1][:, xc], start=st, stop=sp)
                        nc.tensor.matmul(g3, xdrv[3][bt][:, xc], et[2][:, xc], start=st, stop=sp)
                        nc.tensor.matmul(g1, xin[2][bt][:, xc], et[4][:, xc], start=st, stop=sp)
                    # Drain: DVE/ACT read at most ONE PSUM operand per op; g1
                    # staged via SBUF. Scalar (no queued DMAs ahead of it)
                    # copies g3/y0/y2 out of PSUM and issues both out-DMAs;
                    # DVE does the Gauss combine.
                    v_ = vp.tile([128, YCS], f32, tag="v")
                    g3s = vp.tile([128, YCS], f32, tag="g3s")
                    ol = op_.tile([128, 2, YCS], bf16, tag="ol")
                    oh = op_.tile([128, 2, YCS], bf16, tag="oh")
                    nc.scalar.copy(out=g3s, in_=g3)                      # frees g3
                    nc.scalar.copy(out=ol[:, 0], in_=y0)                 # frees y0
                    nc.scalar.copy(out=ol[:, 1], in_=y2)                 # frees y2
                    nc.vector.tensor_copy(out=v_, in_=g1)                # frees g1
                    nc.vector.tensor_sub(out=oh[:, 0], in0=v_, in1=g3s)  # Y1r
                    nc.vector.tensor_add(out=oh[:, 1], in0=v_, in1=g2)   # Y1i, frees g2
                    bsl = slice(bt * 128, (bt + 1) * 128)
                    ysl = ys_d[:, bsl, yc * YCS:(yc + 1) * YCS]
                    nc.scalar.dma_start(
                        out=ysl[0:2].rearrange("s p y -> p s y"), in_=ol)
                    nc.sync.dma_start(
                        out=ysl[2:4].rearrange("s p y -> p s y"), in_=oh)
    nc.compile()
    return nc


def _prep_eigens(eigens):
    """eigens (gy, gx, 4) -> four (YCN, XC, 128, YCS) bf16 chunked E-matrices
    (E0, E1r, E1i, E2), transposed to [x, y] with irfft scales folded in."""
    e = np.ascontiguousarray(eigens.transpose(1, 0, 2)).astype(np.float32)  # (x, y, j)
    e0 = ((e[..., 0] + e[..., 2]) + (e[..., 1] + e[..., 3])) * 0.25
    e2 = ((e[..., 0] + e[..., 2]) - (e[..., 1] + e[..., 3])) * 0.25
    e1r = (e[..., 0] - e[..., 2]) * 0.5
    e1i = (e[..., 3] - e[..., 1]) * 0.5

    import ml_dtypes

    def chunk(m):  # (GX, GY) -> (YCN, XC, 128, YCS)
        return np.ascontiguousarray(
            m.reshape(XC, 128, YCN, YCS).transpose(2, 0, 1, 3)).astype(ml_dtypes.bfloat16)
    return chunk(e0), chunk(e1r), chunk(e1i), chunk(e2)


def _prep_x(x):
    """x (B, IN) f32 -> [4, GX, B] bf16 pre-butterfly streams (transposed)."""
    import ml_dtypes
    xT = np.ascontiguousarray(np.asarray(x, dtype=np.float32).T)  # [IN, B]
    xb = xT.reshape(GX, BLK, B)
    x0, x1, x2, x3 = xb[:, 0], xb[:, 1], xb[:, 2], xb[:, 3]
    xs = np.stack([x0 + x2, x1 + x3, x0 - x2, x3 - x1])  # s02, s13, X1r, X1i
    return xs.astype(ml_dtypes.bfloat16)


def _in_maps(x, eigens):
    e0, e1r, e1i, e2 = _prep_eigens(np.asarray(eigens))
    xs = _prep_x(x)  # [4, GX, B]
    # per-core, packed per b-tile: [s, bt, p(gx%128), xc, b]
    xs = xs.reshape(4, XC, 128, NCORES, BT, 128)
    return [
        {"xs": np.ascontiguousarray(xs[:, :, :, c].transpose(0, 3, 2, 1, 4)),
         "e0": e0, "e1r": e1r, "e1i": e1i, "e2": e2}
        for c in range(NCORES)
    ]


def _combine(ys_list):
    """Per-core [4, BS, GY] bf16 Y-streams -> full (B, OUT) f32 output."""
    ys = np.concatenate([np.asarray(y).astype(np.float32) for y in ys_list],
                        axis=1)  # [4, B, GY]: Y0, Y2, Y1r, Y1i
    a = ys[0] + ys[1]
    b = ys[0] - ys[1]
    out = np.empty((B, GY, BLK), dtype=np.float32)
    out[..., 0] = a + ys[2]
    out[..., 1] = b - ys[3]
    out[..., 2] = a - ys[2]
    out[..., 3] = b + ys[3]
    return out.reshape(B, OUT)


def kernel(x, eigens):
    from concourse.bass_utils import run_bass_kernel_spmd

    if "nc" not in _cache:
        _cache["nc"] = _build_nc()
    res = run_bass_kernel_spmd(_cache["nc"], _in_maps(x, eigens),
                               core_ids=list(range(NCORES)))
    return _combine([r["ys"] for r in res.results])
